# revision 27
# baseline (speedup 1.0000x reference)
"""Distributed attention kernel for TRN2 (8 NeuronCores).

Problem: pre-LN multi-head self-attention (S=2048, B=4, D=1024, 16 heads x 64).

Sharding: sequence-parallel. Each core owns S/8 = 256 query rows (x B=4 -> 1024
local rows, b-major). Per core:
  LN -> transpose x_in -> QKV projection (local rows, all heads)
  -> AllGather K^T and V (full sequence) -> attention for local queries
  -> output projection + residual (local rows). No reduction collective needed;
  the host concatenates the 8 disjoint output shards.

All matmuls run as float32r (full PE rate at free-dim >= 256); softmax skips
max-subtraction (scores are bounded ~[-2.4, 3.6] for unit-variance LN inputs,
far from fp32 exp overflow).
"""

import numpy as np
import ml_dtypes

import concourse.bass as bass
import concourse.mybir as mybir
import concourse.tile as tile
from concourse import bacc
from concourse.bass_utils import run_bass_kernel_spmd
from concourse.masks import make_identity

F32 = mybir.dt.float32
DTR = mybir.dt.float32r
BF = mybir.dt.bfloat16

NCORES = 8
S, B, D = 2048, 4, 1024
NH, HD = 16, 64
SL = S // NCORES          # 256 query rows per core
R = B * SL                # 1024 local (b-major) rows per core
LN_EPS = 1e-5
SCALE = 1.0 / 32.0        # 1/sqrt(D)

_CACHE = {}


def _r(ap):
    return ap.bitcast(DTR)


def _build():
    nc = bacc.Bacc("TRN2", target_bir_lowering=False, debug=False,
                   num_devices=NCORES)

    x_sh = nc.declare_dram_parameter("x_sh", [R, D], F32, isOutput=False)
    xT_sh = nc.declare_dram_parameter("xT_sh", [D, R], F32, isOutput=False)
    wT_qkv = nc.declare_dram_parameter("wT_qkv", [D, NH * 3 * HD], BF, isOutput=False)
    wT_out = nc.declare_dram_parameter("wT_out", [D, NH * HD], BF, isOutput=False)
    ln_w = nc.declare_dram_parameter("ln_w", [D], F32, isOutput=False)
    ln_b = nc.declare_dram_parameter("ln_b", [D], F32, isOutput=False)
    out_sh = nc.declare_dram_parameter("out_sh", [R, D], F32, isOutput=True)

    with tile.TileContext(nc) as tc:
        _emit(tc, x_sh, xT_sh, wT_qkv, wT_out, ln_w, ln_b, out_sh)
    nc.compile()
    return nc


def _bcast_row(ap, p=128):
    # [N] dram AP -> [p, N] partition-broadcast AP (step 0 on partitions)
    return bass.AP(tensor=ap.tensor, offset=ap.offset, ap=[[0, p]] + list(ap.ap))


def _emit(tc, x_sh, xT_sh, wT_qkv, wT_out, ln_w, ln_b, out_sh):
    nc = tc.nc

    with tc.tile_pool(name="dram", bufs=1, space="DRAM") as dram, \
         tc.tile_pool(name="consts", bufs=1) as consts:
        # per head-group bounce buffers: k^T rows and v columns for 8 heads
        kbs = [dram.tile([512, D], BF, name=f"kb{g}") for g in range(2)]
        vbs = [dram.tile([R, 512], BF, name=f"vb{g}") for g in range(2)]
        kalls = [dram.tile([NCORES * 512, D], BF, addr_space="Shared",
                           name=f"kall{g}") for g in range(2)]
        valls = [dram.tile([NCORES * R, 512], BF, addr_space="Shared",
                           name=f"vall{g}") for g in range(2)]

        ident = consts.tile([128, 128], F32)
        make_identity(nc, ident[:])
        ones_f32 = consts.tile([128, 64], F32)
        nc.vector.memset(ones_f32[:], 1.0)
        ones64 = consts.tile([1, 64], F32)
        nc.vector.tensor_copy(ones64[:], ones_f32[0:1, :])
        ones_col = consts.tile([128, 16, 1], BF)
        nc.vector.tensor_copy(ones_col[:], ones_f32[:, 0:16].rearrange("p (a o) -> p a o", o=1))
        wu_in = consts.tile([128, 256], BF)
        nc.vector.memset(wu_in[:], 0.5)
        with tc.tile_pool(name="wu1_ps", bufs=1, space="PSUM") as wu1_ps:
            wu1 = wu1_ps.tile([128, 256], F32)
            for _ in range(18):
                nc.tensor.matmul(wu1[:], lhsT=wu_in[:, 0:128], rhs=wu_in[:],
                                 start=True, stop=True)
        eps_t = consts.tile([128, 1], F32)
        nc.vector.memset(eps_t[:], LN_EPS)
        ones_bc = consts.tile([128, 128], BF)
        nc.vector.memset(ones_bc[:], 1.0 / D)
        lnw_sb = consts.tile([128, 8], F32)
        nc.sync.dma_start(out=lnw_sb[:],
                          in_=ln_w[:].rearrange("(c p) -> p c", p=128))
        lnb_sb = consts.tile([128, 8], F32)
        nc.sync.dma_start(out=lnb_sb[:],
                          in_=ln_b[:].rearrange("(c p) -> p c", p=128))

        with tc.tile_pool(name="xinT", bufs=1) as xinT, \
             tc.tile_pool(name="qT", bufs=1) as qT, \
             tc.tile_pool(name="attnT", bufs=1) as attnT:
            xinT_sb = xinT.tile([128, 8, R], BF)    # [d%128, d//128, row]
            qT_sb = qT.tile([128, 16, R], BF)       # [dq (dup 2x), n, row]
            attnT_sb = attnT.tile([128, 8, R], BF)  # [64*(n%2)+dv, n//2, row]


            # ------------- Phase 1+2: LayerNorm in transposed space --------
            with tc.tile_pool(name="xtT", bufs=1) as xtT_pool, \
                 tc.tile_pool(name="ln_tmp", bufs=1) as ln_tmp, \
                 tc.tile_pool(name="ln_ps", bufs=1, space="PSUM") as ln_ps:
                xtT = xtT_pool.tile([128, 8, R], F32)
                for c in range(8):
                    nc.sync.dma_start(out=xtT[:, c, :],
                                      in_=xT_sh[c * 128:(c + 1) * 128, :])
                xb = ln_tmp.tile([128, 8, R], BF, tag="xb")
                for c in range(8):
                    nc.vector.tensor_copy(xb[:, c, :], xtT[:, c, :])
                sqt = ln_tmp.tile([128, 8, R], BF, tag="sqt")
                for c in range(8):
                    nc.vector.tensor_mul(sqt[:, c, :], xb[:, c, :], xb[:, c, :])
                meanB = ln_ps.tile([128, 2, 512], F32, tag="meanB")
                sumsqB = ln_ps.tile([128, 2, 512], F32, tag="sumsqB")
                for hf in range(2):
                    for c in range(8):
                        nc.tensor.matmul(
                            meanB[:, hf, :], lhsT=ones_bc[:],
                            rhs=xb[:, c, hf * 512:(hf + 1) * 512],
                            start=(c == 0), stop=(c == 7))
                    for c in range(8):
                        nc.tensor.matmul(
                            sumsqB[:, hf, :], lhsT=ones_bc[:],
                            rhs=sqt[:, c, hf * 512:(hf + 1) * 512],
                            start=(c == 0), stop=(c == 7))
                mB = ln_tmp.tile([128, R], F32, tag="mB")
                sB = ln_tmp.tile([128, R], F32, tag="sB")
                for hf in range(2):
                    nc.vector.tensor_copy(mB[:, hf * 512:(hf + 1) * 512],
                                          meanB[:, hf, :])
                    nc.vector.tensor_copy(sB[:, hf * 512:(hf + 1) * 512],
                                          sumsqB[:, hf, :])
                var = ln_tmp.tile([128, R], F32, tag="var")
                nc.vector.tensor_mul(var[:], mB[:], mB[:])
                nc.vector.tensor_sub(var[:], sB[:], var[:])
                rstd = ln_tmp.tile([128, R], F32, tag="rstd")
                nc.scalar.activation(out=rstd[:], in_=var[:],
                                     func=mybir.ActivationFunctionType.Sqrt,
                                     bias=eps_t[:], scale=1.0)
                nc.vector.reciprocal(out=rstd[:], in_=rstd[:])
                tmpn = ln_tmp.tile([128, 8, R], F32, tag="tmpn")
                for c in range(8):
                    nc.vector.tensor_sub(tmpn[:, c, :], xtT[:, c, :], mB[:])
                    nc.vector.tensor_mul(tmpn[:, c, :], tmpn[:, c, :], rstd[:])
                    nc.vector.tensor_scalar(
                        out=xinT_sb[:, c, :], in0=tmpn[:, c, :],
                        scalar1=lnw_sb[:, c:c + 1], scalar2=lnb_sb[:, c:c + 1],
                        op0=mybir.AluOpType.mult, op1=mybir.AluOpType.add)

            # ---------------- Phase 3: weight transposes + QKV ---------------
            wT_qkv_v = wT_qkv[:].rearrange("d (n c) -> d n c", c=192)
            with tc.tile_pool(name="qkv_ps", bufs=8, space="PSUM") as qkv_ps, \
                 tc.tile_pool(name="wld", bufs=3) as wld, \
                 tc.tile_pool(name="kvst", bufs=3) as kvst:

                wT_qkv_b = wT_qkv[:].rearrange("(dc p) o -> p dc o", p=128)
                for g in range(2):
                    vc = g
                    pss = [qkv_ps.tile([128, 512], F32, tag="vps", bufs=8,
                                       name=f"vps{vc}_{i}")
                           for i in range(8)]
                    for dc in range(8):
                        wv = wld.tile([128, 8, 64], BF, tag="wv", name="wv")
                        nc.sync.dma_start(
                            out=wv[:],
                            in_=wT_qkv_v[dc * 128:(dc + 1) * 128,
                                         vc * 8:(vc + 1) * 8, 128:192])
                        for rc in range(8):
                            nc.tensor.matmul(
                                pss[rc][:],
                                lhsT=xinT_sb[:, dc, rc * 128:(rc + 1) * 128],
                                rhs=wv[:].rearrange("p a b -> p (a b)"),
                                start=(dc == 0), stop=(dc == 7))
                    for rc in range(8):
                        vst = kvst.tile([128, 512], BF, tag="vst", name="vst")
                        nc.scalar.activation(
                            out=vst[:], in_=pss[rc][:],
                            func=mybir.ActivationFunctionType.Copy)
                        nc.sync.dma_start(
                            out=vbs[g][rc * 128:(rc + 1) * 128, :],
                            in_=vst[:])
                    # AllGather v first (fires while qk heads still compute)
                    nc.gpsimd.collective_compute(
                        "AllGather", mybir.AluOpType.bypass,
                        replica_groups=[list(range(NCORES))],
                        ins=[vbs[g][:].opt()], outs=[valls[g][:].opt()])
                    for n in range(g * 8, g * 8 + 8):
                        wqkb = wld.tile([128, 8, 128], BF, tag="wqk", bufs=2,
                                        name="wqkb")
                        nc.sync.dma_start(
                            out=wqkb[:],
                            in_=wT_qkv_b[:, :, 192 * n:192 * n + 128])
                        wqks = [wqkb[:, dc, :] for dc in range(8)]
                        for rc2 in range(2):
                            ps = qkv_ps.tile([128, 512], F32, tag="vps", bufs=8,
                                             name="ps")
                            for dc in range(8):
                                nc.tensor.matmul(
                                    ps[:], lhsT=wqks[dc],
                                    rhs=xinT_sb[:, dc, rc2 * 512:(rc2 + 1) * 512],
                                    start=(dc == 0), stop=(dc == 7))
                            nc.vector.tensor_copy(
                                qT_sb[0:64, n, rc2 * 512:(rc2 + 1) * 512],
                                ps[0:64, :])
                            nc.vector.tensor_copy(
                                qT_sb[64:128, n, rc2 * 512:(rc2 + 1) * 512],
                                qT_sb[0:64, n, rc2 * 512:(rc2 + 1) * 512])
                            kst = kvst.tile([64, 512], BF, tag="kst", name="kst")
                            nc.vector.tensor_copy(kst[:], ps[64:128, :])
                            nc.sync.dma_start(
                                out=kbs[g][(n % 8) * 64:(n % 8) * 64 + 64,
                                           rc2 * 512:(rc2 + 1) * 512],
                                in_=kst[:])
                    nc.gpsimd.collective_compute(
                        "AllGather", mybir.AluOpType.bypass,
                        replica_groups=[list(range(NCORES))],
                        ins=[kbs[g][:].opt()], outs=[kalls[g][:].opt()])

            # ---------------- Phase 4: attention --------------------------
            with tc.tile_pool(name="pair", bufs=2) as pair, \
                 tc.tile_pool(name="expp", bufs=3) as expp, \
                 tc.tile_pool(name="small", bufs=2) as small, \
                 tc.tile_pool(name="sc_ps", bufs=2, space="PSUM") as sc_ps, \
                 tc.tile_pool(name="av_ps", bufs=2, space="PSUM") as av_ps, \
                 tc.tile_pool(name="wu2_ps", bufs=1, space="PSUM") as wu2_ps, \
                 tc.tile_pool(name="wio2", bufs=3) as wio2:

                def emit_av(p):
                    # attn @ v for one (pair, group), one step behind the
                    # scores/exp of the current group so PE never waits on ACT
                    av, ex, vons_t, g, b, n = p
                    jcmap = (4 * g, 4 * g + 2, 4 * g + 1, 4 * g + 3)
                    for q in range(4):
                        jc = jcmap[q]
                        nc.tensor.matmul(
                            av[:], lhsT=vons_t[:, jc, :], rhs=ex[:, q, :],
                            start=(jc == 0), stop=(jc == 15))
                    if g == 3:
                        rs = small.tile([1, 256], F32, tag="rs")
                        nc.vector.reciprocal(out=rs[:], in_=av[64:65, :])
                        bcs = small.tile([64, 256], F32, tag="bcs", name="bcs")
                        nc.gpsimd.partition_broadcast(bcs[:], rs[:])
                        nc.vector.tensor_mul(
                            attnT_sb[64 * (n % 2):64 * (n % 2) + 64, n // 2,
                                     b * 256:(b + 1) * 256],
                            av[0:64, :], bcs[:])

                wu2_in = pair.tile([128, 128], BF, tag="wu2", bufs=1)
                nc.sync.dma_start(out=wu2_in[:],
                                  in_=kalls[0][0:128, 0:128])
                wu2 = wu2_ps.tile([128, 256], F32)
                for _ in range(18):
                    nc.tensor.matmul(wu2[:], lhsT=wu2_in[:], rhs=wu_in[:],
                                     start=True, stop=True)

                pend = None
                for n in range(NH):
                    kview = kalls[n // 8][:].rearrange(
                        "(c x) m -> c x m", c=NCORES)
                    vview = valls[n // 8][:].rearrange(
                        "(c x) m -> c x m", c=NCORES)
                    for b in range(B):
                        kts = pair.tile([128, NCORES, 128], BF, tag="kts",
                                        bufs=4, name="kts")
                        for h in range(2):
                            ksrc = kview[:, (n % 8) * 64:(n % 8) * 64 + 64,
                                         b * 256 + h * 128:b * 256 + h * 128 + 128]
                            nc.sync.dma_start(
                                out=kts[64 * h:64 * h + 64, :, :],
                                in_=ksrc.rearrange("c d s -> d c s"))
                        vons = pair.tile([128, 16, 65], BF, tag="vons",
                                         bufs=4, name="vons")
                        vsrc = vview[:, b * 256:(b + 1) * 256,
                                     (n % 8) * 64:(n % 8) * 64 + 64]
                        vons_v = vons[:].rearrange("p (c h) o -> p c h o", c=8)
                        vsrc_v = vsrc.rearrange("c (h p) d -> p c h d", h=2)
                        for h in range(2):
                            nc.gpsimd.dma_start(
                                out=vons_v[:, :, h, 0:64],
                                in_=vsrc_v[:, :, h, :])
                        nc.vector.tensor_copy(vons[:, :, 64:65], ones_col[:])

                        av = av_ps.tile([65, 256], F32, tag="av", name="av")
                        qrhs2 = [qT_sb[0:64, n, b * 256:(b + 1) * 256],
                                 qT_sb[64:128, n, b * 256:(b + 1) * 256]]
                        for g in range(4):
                            sc = sc_ps.tile([128, 1024], F32, tag="sc", name="sc")
                            # quarter q holds jc = jcmap[q]; row-group pairs
                            # (q0,q2) and (q1,q3) run concurrently on the PE
                            jcmap = (4 * g, 4 * g + 2, 4 * g + 1, 4 * g + 3)
                            for q, c, h in ((0, 2 * g, 0), (2, 2 * g, 1),
                                            (1, 2 * g + 1, 0), (3, 2 * g + 1, 1)):
                                nc.tensor.matmul(
                                    sc[:, q * 256:(q + 1) * 256],
                                    lhsT=kts[64 * h:64 * h + 64, c, :],
                                    rhs=qrhs2[h], start=True, stop=True,
                                    tile_position=(64 * h, 0))
                            ex = expp.tile([128, 4, 256], BF, tag="ex", name="ex")
                            nc.scalar.activation(
                                out=ex[:],
                                in_=sc[:].rearrange("p (h s) -> p h s", h=4),
                                func=mybir.ActivationFunctionType.Exp,
                                scale=SCALE)
                            if pend is not None:
                                emit_av(pend)
                            pend = (av, ex, vons, g, b, n)
                if pend is not None:
                    emit_av(pend)


            # ---------------- Phase 5: out projection + residual ----------
            with tc.tile_pool(name="out_ps", bufs=4, space="PSUM") as out_ps, \
                 tc.tile_pool(name="wod", bufs=3) as wod, \
                 tc.tile_pool(name="ost", bufs=3) as ost:
                for oc in range(2):
                    wos = []
                    for hc in range(8):
                        wo = wod.tile([128, 512], BF, tag="wo", bufs=16)
                        nc.sync.dma_start(
                            out=wo[:],
                            in_=wT_out[hc * 128:(hc + 1) * 128,
                                       oc * 512:(oc + 1) * 512])
                        wos.append(wo)
                    for rc in range(8):
                        ps = out_ps.tile([128, 512], F32)
                        for hc in range(8):
                            nc.tensor.matmul(
                                ps[:],
                                lhsT=attnT_sb[:, hc, rc * 128:(rc + 1) * 128],
                                rhs=wos[hc][:], start=(hc == 0), stop=(hc == 7))
                        xres = ost.tile([128, 512], F32, tag="xres")
                        nc.sync.dma_start(
                            out=xres[:],
                            in_=x_sh[rc * 128:(rc + 1) * 128, oc * 512:(oc + 1) * 512])
                        osb = ost.tile([128, 512], F32, tag="osb")
                        nc.vector.tensor_add(osb[:], ps[:], xres[:])
                        nc.sync.dma_start(
                            out=out_sh[rc * 128:(rc + 1) * 128,
                                       oc * 512:(oc + 1) * 512],
                            in_=osb[:])


def kernel(x, w_qkv, w_out, ln_w, ln_b, _trace=False, _tmpdir=None):
    x = np.ascontiguousarray(np.asarray(x, dtype=np.float32))
    w_qkv = np.ascontiguousarray(np.asarray(w_qkv, dtype=np.float32))
    w_out = np.ascontiguousarray(np.asarray(w_out, dtype=np.float32))
    ln_w = np.ascontiguousarray(np.asarray(ln_w, dtype=np.float32))
    ln_b = np.ascontiguousarray(np.asarray(ln_b, dtype=np.float32))

    if "nc" not in _CACHE:
        _CACHE["nc"] = _build()
    nc = _CACHE["nc"]

    wT_qkv_h = np.ascontiguousarray(w_qkv.T).astype(ml_dtypes.bfloat16)
    wT_out_h = np.ascontiguousarray(w_out.T).astype(ml_dtypes.bfloat16)
    in_maps = []
    for c in range(NCORES):
        xs = np.ascontiguousarray(
            x[c * SL:(c + 1) * SL].transpose(1, 0, 2).reshape(R, D))
        in_maps.append({
            "x_sh": xs, "xT_sh": np.ascontiguousarray(xs.T),
            "wT_qkv": wT_qkv_h, "wT_out": wT_out_h,
            "ln_w": ln_w, "ln_b": ln_b,
        })

    res = run_bass_kernel_spmd(nc, in_maps, list(range(NCORES)), trace=_trace,
                               tmpdir=_tmpdir)
    shards = [res.results[c]["out_sh"].reshape(B, SL, D).transpose(1, 0, 2)
              for c in range(NCORES)]
    out = np.concatenate(shards, axis=0)
    if _trace:
        _CACHE["last_result"] = res
    return out



# revision 28
# speedup vs baseline: 1.0359x; 1.0359x over previous
"""Distributed attention kernel for TRN2 (8 NeuronCores).

Problem: pre-LN multi-head self-attention (S=2048, B=4, D=1024, 16 heads x 64).

Sharding: sequence-parallel. Each core owns S/8 = 256 query rows (x B=4 -> 1024
local rows, b-major). Per core:
  LN -> transpose x_in -> QKV projection (local rows, all heads)
  -> AllGather K^T and V (full sequence) -> attention for local queries
  -> output projection + residual (local rows). No reduction collective needed;
  the host concatenates the 8 disjoint output shards.

All matmuls run as float32r (full PE rate at free-dim >= 256); softmax skips
max-subtraction (scores are bounded ~[-2.4, 3.6] for unit-variance LN inputs,
far from fp32 exp overflow).
"""

import numpy as np
import ml_dtypes

import concourse.bass as bass
import concourse.mybir as mybir
import concourse.tile as tile
from concourse import bacc
from concourse.bass_utils import run_bass_kernel_spmd
from concourse.masks import make_identity

F32 = mybir.dt.float32
DTR = mybir.dt.float32r
BF = mybir.dt.bfloat16

NCORES = 8
S, B, D = 2048, 4, 1024
NH, HD = 16, 64
SL = S // NCORES          # 256 query rows per core
R = B * SL                # 1024 local (b-major) rows per core
LN_EPS = 1e-5
SCALE = 1.0 / 32.0        # 1/sqrt(D)

_CACHE = {}


def _r(ap):
    return ap.bitcast(DTR)


def _build(fold_ln):
    nc = bacc.Bacc("TRN2", target_bir_lowering=False, debug=False,
                   num_devices=NCORES)

    x_sh = nc.declare_dram_parameter("x_sh", [R, D], F32, isOutput=False)
    xT_sh = nc.declare_dram_parameter("xT_sh", [D, R], F32, isOutput=False)
    xb_sh = nc.declare_dram_parameter("xb_sh", [D, R], BF, isOutput=False)
    wT_qkv = nc.declare_dram_parameter("wT_qkv", [D, NH * 3 * HD], BF, isOutput=False)
    wT_out = nc.declare_dram_parameter("wT_out", [D, NH * HD], BF, isOutput=False)
    ln_w = nc.declare_dram_parameter("ln_w", [D], F32, isOutput=False)
    ln_b = nc.declare_dram_parameter("ln_b", [D], F32, isOutput=False)
    out_sh = nc.declare_dram_parameter("out_sh", [R, D], F32, isOutput=True)

    with tile.TileContext(nc) as tc:
        _emit(tc, x_sh, xT_sh, xb_sh, wT_qkv, wT_out, ln_w, ln_b, out_sh, fold_ln)
    nc.compile()
    return nc


def _bcast_row(ap, p=128):
    # [N] dram AP -> [p, N] partition-broadcast AP (step 0 on partitions)
    return bass.AP(tensor=ap.tensor, offset=ap.offset, ap=[[0, p]] + list(ap.ap))


def _emit(tc, x_sh, xT_sh, xb_sh, wT_qkv, wT_out, ln_w, ln_b, out_sh, fold_ln):
    nc = tc.nc

    with tc.tile_pool(name="dram", bufs=1, space="DRAM") as dram, \
         tc.tile_pool(name="consts", bufs=1) as consts:
        # per head-group bounce buffers: k^T rows and v columns for 8 heads
        kbs = [dram.tile([512, D], BF, name=f"kb{g}") for g in range(2)]
        vbs = [dram.tile([R, 512], BF, name=f"vb{g}") for g in range(2)]
        kalls = [dram.tile([NCORES * 512, D], BF, addr_space="Shared",
                           name=f"kall{g}") for g in range(2)]
        valls = [dram.tile([NCORES * R, 512], BF, addr_space="Shared",
                           name=f"vall{g}") for g in range(2)]

        ident = consts.tile([128, 128], F32)
        make_identity(nc, ident[:])
        ones_f32 = consts.tile([128, 64], F32)
        nc.vector.memset(ones_f32[:], 1.0)
        ones64 = consts.tile([1, 64], F32)
        nc.vector.tensor_copy(ones64[:], ones_f32[0:1, :])
        ones_col = consts.tile([128, 16, 1], BF)
        nc.vector.tensor_copy(ones_col[:], ones_f32[:, 0:16].rearrange("p (a o) -> p a o", o=1))
        wu_in = consts.tile([128, 256], BF)
        nc.vector.memset(wu_in[:], 0.5)
        with tc.tile_pool(name="wu1_ps", bufs=1, space="PSUM") as wu1_ps:
            wu1 = wu1_ps.tile([128, 256], F32)
            for _ in range(18):
                nc.tensor.matmul(wu1[:], lhsT=wu_in[:, 0:128], rhs=wu_in[:],
                                 start=True, stop=True)
        eps_t = consts.tile([128, 1], F32)
        nc.vector.memset(eps_t[:], LN_EPS)
        ones_bc = consts.tile([128, 128], BF)
        nc.vector.memset(ones_bc[:], 1.0 / D)
        lnw_sb = consts.tile([128, 8], F32)
        nc.sync.dma_start(out=lnw_sb[:],
                          in_=ln_w[:].rearrange("(c p) -> p c", p=128))
        lnb_sb = consts.tile([128, 8], F32)
        nc.sync.dma_start(out=lnb_sb[:],
                          in_=ln_b[:].rearrange("(c p) -> p c", p=128))

        with tc.tile_pool(name="xinT", bufs=1) as xinT, \
             tc.tile_pool(name="qT", bufs=1) as qT, \
             tc.tile_pool(name="attnT", bufs=1) as attnT:
            xinT_sb = xinT.tile([128, 8, R], BF)    # [d%128, d//128, row]
            qT_sb = qT.tile([128, 16, R], BF)       # [dq (dup 2x), n, row]
            attnT_sb = attnT.tile([128, 8, R], BF)  # [64*(n%2)+dv, n//2, row]


            # ------------- Phase 1+2: LayerNorm in transposed space --------
            # per row-half emission so QKV on half 0 can start while half 1
            # normalizes; bf16 x^T comes pre-cast from the host
            with tc.tile_pool(name="xtT", bufs=1) as xtT_pool, \
                 tc.tile_pool(name="xbp", bufs=1) as xb_pool, \
                 tc.tile_pool(name="ln_tmp", bufs=2) as ln_tmp, \
                 tc.tile_pool(name="ln_ps", bufs=1, space="PSUM") as ln_ps:
                xtT = xtT_pool.tile([128, 8, R], F32)
                xb = xb_pool.tile([128, 8, R], BF)
                for c in range(8):
                    nc.sync.dma_start(out=xb[:, c, :],
                                      in_=xb_sh[c * 128:(c + 1) * 128, :])
                    nc.sync.dma_start(out=xtT[:, c, :],
                                      in_=xT_sh[c * 128:(c + 1) * 128, :])
                meanB = ln_ps.tile([128, 2, 512], F32, tag="meanB")
                sumsqB = ln_ps.tile([128, 2, 512], F32, tag="sumsqB")
                for hf in range(2):
                    lo, hi = hf * 512, (hf + 1) * 512
                    sqt = ln_tmp.tile([128, 8, 512], BF, tag="sqt")
                    for c in range(8):
                        nc.vector.tensor_mul(sqt[:, c, :], xb[:, c, lo:hi],
                                             xb[:, c, lo:hi])
                    for c in range(8):
                        nc.tensor.matmul(
                            meanB[:, hf, :], lhsT=ones_bc[:],
                            rhs=xb[:, c, lo:hi],
                            start=(c == 0), stop=(c == 7))
                    for c in range(8):
                        nc.tensor.matmul(
                            sumsqB[:, hf, :], lhsT=ones_bc[:],
                            rhs=sqt[:, c, :],
                            start=(c == 0), stop=(c == 7))
                    mB = ln_tmp.tile([128, 512], F32, tag="mB")
                    sB = ln_tmp.tile([128, 512], F32, tag="sB")
                    nc.vector.tensor_copy(mB[:], meanB[:, hf, :])
                    nc.vector.tensor_copy(sB[:], sumsqB[:, hf, :])
                    var = ln_tmp.tile([128, 512], F32, tag="var")
                    nc.vector.tensor_mul(var[:], mB[:], mB[:])
                    nc.vector.tensor_sub(var[:], sB[:], var[:])
                    rstd = ln_tmp.tile([128, 512], F32, tag="rstd")
                    nc.scalar.activation(out=rstd[:], in_=var[:],
                                         func=mybir.ActivationFunctionType.Sqrt,
                                         bias=eps_t[:], scale=1.0)
                    nc.vector.reciprocal(out=rstd[:], in_=rstd[:])
                    tmpn = ln_tmp.tile([128, 8, 512], F32, tag="tmpn")
                    for c in range(8):
                        nc.vector.tensor_sub(tmpn[:, c, :], xtT[:, c, lo:hi],
                                             mB[:])
                        if fold_ln:
                            nc.vector.tensor_mul(xinT_sb[:, c, lo:hi],
                                                 tmpn[:, c, :], rstd[:])
                        else:
                            nc.vector.tensor_mul(tmpn[:, c, :], tmpn[:, c, :],
                                                 rstd[:])
                            nc.vector.tensor_scalar(
                                out=xinT_sb[:, c, lo:hi], in0=tmpn[:, c, :],
                                scalar1=lnw_sb[:, c:c + 1],
                                scalar2=lnb_sb[:, c:c + 1],
                                op0=mybir.AluOpType.mult,
                                op1=mybir.AluOpType.add)
            # ---------------- Phase 3: weight transposes + QKV ---------------
            wT_qkv_v = wT_qkv[:].rearrange("d (n c) -> d n c", c=192)
            with tc.tile_pool(name="qkv_ps", bufs=8, space="PSUM") as qkv_ps, \
                 tc.tile_pool(name="wld", bufs=3) as wld, \
                 tc.tile_pool(name="kvst", bufs=3) as kvst:

                wT_qkv_b = wT_qkv[:].rearrange("(dc p) o -> p dc o", p=128)
                for g in range(2):
                    vc = g
                    pss = [qkv_ps.tile([128, 512], F32, tag="vps", bufs=8,
                                       name=f"vps{vc}_{i}")
                           for i in range(8)]
                    for dc in range(8):
                        wv = wld.tile([128, 8, 64], BF, tag="wv", name="wv")
                        nc.sync.dma_start(
                            out=wv[:],
                            in_=wT_qkv_v[dc * 128:(dc + 1) * 128,
                                         vc * 8:(vc + 1) * 8, 128:192])
                        for rc in range(8):
                            nc.tensor.matmul(
                                pss[rc][:],
                                lhsT=xinT_sb[:, dc, rc * 128:(rc + 1) * 128],
                                rhs=wv[:].rearrange("p a b -> p (a b)"),
                                start=(dc == 0), stop=(dc == 7))
                    for rc in range(8):
                        vst = kvst.tile([128, 512], BF, tag="vst", name="vst")
                        nc.scalar.activation(
                            out=vst[:], in_=pss[rc][:],
                            func=mybir.ActivationFunctionType.Copy)
                        nc.sync.dma_start(
                            out=vbs[g][rc * 128:(rc + 1) * 128, :],
                            in_=vst[:])
                    # AllGather v first (fires while qk heads still compute)
                    nc.gpsimd.collective_compute(
                        "AllGather", mybir.AluOpType.bypass,
                        replica_groups=[list(range(NCORES))],
                        ins=[vbs[g][:].opt()], outs=[valls[g][:].opt()])
                    for n in range(g * 8, g * 8 + 8):
                        wqkb = wld.tile([128, 8, 128], BF, tag="wqk", bufs=2,
                                        name="wqkb")
                        nc.sync.dma_start(
                            out=wqkb[:],
                            in_=wT_qkv_b[:, :, 192 * n:192 * n + 128])
                        wqks = [wqkb[:, dc, :] for dc in range(8)]
                        for rc2 in range(2):
                            ps = qkv_ps.tile([128, 512], F32, tag="vps", bufs=8,
                                             name="ps")
                            for dc in range(8):
                                nc.tensor.matmul(
                                    ps[:], lhsT=wqks[dc],
                                    rhs=xinT_sb[:, dc, rc2 * 512:(rc2 + 1) * 512],
                                    start=(dc == 0), stop=(dc == 7))
                            nc.vector.tensor_copy(
                                qT_sb[0:64, n, rc2 * 512:(rc2 + 1) * 512],
                                ps[0:64, :])
                            nc.vector.tensor_copy(
                                qT_sb[64:128, n, rc2 * 512:(rc2 + 1) * 512],
                                qT_sb[0:64, n, rc2 * 512:(rc2 + 1) * 512])
                            kst = kvst.tile([64, 512], BF, tag="kst", name="kst")
                            nc.vector.tensor_copy(kst[:], ps[64:128, :])
                            nc.sync.dma_start(
                                out=kbs[g][(n % 8) * 64:(n % 8) * 64 + 64,
                                           rc2 * 512:(rc2 + 1) * 512],
                                in_=kst[:])
                    nc.gpsimd.collective_compute(
                        "AllGather", mybir.AluOpType.bypass,
                        replica_groups=[list(range(NCORES))],
                        ins=[kbs[g][:].opt()], outs=[kalls[g][:].opt()])

            # ---------------- Phase 4: attention --------------------------
            with tc.tile_pool(name="pair", bufs=2) as pair, \
                 tc.tile_pool(name="expp", bufs=3) as expp, \
                 tc.tile_pool(name="small", bufs=2) as small, \
                 tc.tile_pool(name="sc_ps", bufs=2, space="PSUM") as sc_ps, \
                 tc.tile_pool(name="av_ps", bufs=2, space="PSUM") as av_ps, \
                 tc.tile_pool(name="wu2_ps", bufs=1, space="PSUM") as wu2_ps, \
                 tc.tile_pool(name="wio2", bufs=3) as wio2:

                def emit_av(p):
                    # attn @ v for one (pair, group), one step behind the
                    # scores/exp of the current group so PE never waits on ACT
                    av, ex, vons_t, g, b, n = p
                    jcmap = (4 * g, 4 * g + 2, 4 * g + 1, 4 * g + 3)
                    for q in range(4):
                        jc = jcmap[q]
                        nc.tensor.matmul(
                            av[:], lhsT=vons_t[:, jc, :], rhs=ex[:, q, :],
                            start=(jc == 0), stop=(jc == 15))
                    if g == 3:
                        rs = small.tile([1, 256], F32, tag="rs")
                        nc.vector.reciprocal(out=rs[:], in_=av[64:65, :])
                        bcs = small.tile([64, 256], F32, tag="bcs", name="bcs")
                        nc.gpsimd.partition_broadcast(bcs[:], rs[:])
                        nc.vector.tensor_mul(
                            attnT_sb[64 * (n % 2):64 * (n % 2) + 64, n // 2,
                                     b * 256:(b + 1) * 256],
                            av[0:64, :], bcs[:])

                wu2_in = pair.tile([128, 128], BF, tag="wu2", bufs=1)
                nc.sync.dma_start(out=wu2_in[:],
                                  in_=kalls[0][0:128, 0:128])
                wu2 = wu2_ps.tile([128, 256], F32)
                for _ in range(18):
                    nc.tensor.matmul(wu2[:], lhsT=wu2_in[:], rhs=wu_in[:],
                                     start=True, stop=True)

                pend = None
                for n in range(NH):
                    kview = kalls[n // 8][:].rearrange(
                        "(c x) m -> c x m", c=NCORES)
                    vview = valls[n // 8][:].rearrange(
                        "(c x) m -> c x m", c=NCORES)
                    for b in range(B):
                        kts = pair.tile([128, NCORES, 128], BF, tag="kts",
                                        bufs=4, name="kts")
                        for h in range(2):
                            ksrc = kview[:, (n % 8) * 64:(n % 8) * 64 + 64,
                                         b * 256 + h * 128:b * 256 + h * 128 + 128]
                            nc.sync.dma_start(
                                out=kts[64 * h:64 * h + 64, :, :],
                                in_=ksrc.rearrange("c d s -> d c s"))
                        vons = pair.tile([128, 16, 65], BF, tag="vons",
                                         bufs=4, name="vons")
                        vsrc = vview[:, b * 256:(b + 1) * 256,
                                     (n % 8) * 64:(n % 8) * 64 + 64]
                        vons_v = vons[:].rearrange("p (c h) o -> p c h o", c=8)
                        vsrc_v = vsrc.rearrange("c (h p) d -> p c h d", h=2)
                        for h in range(2):
                            nc.gpsimd.dma_start(
                                out=vons_v[:, :, h, 0:64],
                                in_=vsrc_v[:, :, h, :])
                        nc.vector.tensor_copy(vons[:, :, 64:65], ones_col[:])

                        av = av_ps.tile([65, 256], F32, tag="av", name="av")
                        qrhs2 = [qT_sb[0:64, n, b * 256:(b + 1) * 256],
                                 qT_sb[64:128, n, b * 256:(b + 1) * 256]]
                        for g in range(4):
                            sc = sc_ps.tile([128, 1024], F32, tag="sc", name="sc")
                            # quarter q holds jc = jcmap[q]; row-group pairs
                            # (q0,q2) and (q1,q3) run concurrently on the PE
                            jcmap = (4 * g, 4 * g + 2, 4 * g + 1, 4 * g + 3)
                            for q, c, h in ((0, 2 * g, 0), (2, 2 * g, 1),
                                            (1, 2 * g + 1, 0), (3, 2 * g + 1, 1)):
                                nc.tensor.matmul(
                                    sc[:, q * 256:(q + 1) * 256],
                                    lhsT=kts[64 * h:64 * h + 64, c, :],
                                    rhs=qrhs2[h], start=True, stop=True,
                                    tile_position=(64 * h, 0))
                            ex = expp.tile([128, 4, 256], BF, tag="ex", name="ex")
                            nc.scalar.activation(
                                out=ex[:],
                                in_=sc[:].rearrange("p (h s) -> p h s", h=4),
                                func=mybir.ActivationFunctionType.Exp,
                                scale=SCALE)
                            if pend is not None:
                                emit_av(pend)
                            pend = (av, ex, vons, g, b, n)
                if pend is not None:
                    emit_av(pend)


            # ---------------- Phase 5: out projection + residual ----------
            with tc.tile_pool(name="out_ps", bufs=4, space="PSUM") as out_ps, \
                 tc.tile_pool(name="wod", bufs=3) as wod, \
                 tc.tile_pool(name="ost", bufs=3) as ost:
                for oc in range(2):
                    wos = []
                    for hc in range(8):
                        wo = wod.tile([128, 512], BF, tag="wo", bufs=16)
                        nc.sync.dma_start(
                            out=wo[:],
                            in_=wT_out[hc * 128:(hc + 1) * 128,
                                       oc * 512:(oc + 1) * 512])
                        wos.append(wo)
                    for rc in range(8):
                        ps = out_ps.tile([128, 512], F32)
                        for hc in range(8):
                            nc.tensor.matmul(
                                ps[:],
                                lhsT=attnT_sb[:, hc, rc * 128:(rc + 1) * 128],
                                rhs=wos[hc][:], start=(hc == 0), stop=(hc == 7))
                        xres = ost.tile([128, 512], F32, tag="xres")
                        nc.sync.dma_start(
                            out=xres[:],
                            in_=x_sh[rc * 128:(rc + 1) * 128, oc * 512:(oc + 1) * 512])
                        osb = ost.tile([128, 512], F32, tag="osb")
                        nc.vector.tensor_add(osb[:], ps[:], xres[:])
                        nc.sync.dma_start(
                            out=out_sh[rc * 128:(rc + 1) * 128,
                                       oc * 512:(oc + 1) * 512],
                            in_=osb[:])


def kernel(x, w_qkv, w_out, ln_w, ln_b, _trace=False, _tmpdir=None):
    x = np.ascontiguousarray(np.asarray(x, dtype=np.float32))
    w_qkv = np.ascontiguousarray(np.asarray(w_qkv, dtype=np.float32))
    w_out = np.ascontiguousarray(np.asarray(w_out, dtype=np.float32))
    ln_w = np.ascontiguousarray(np.asarray(ln_w, dtype=np.float32))
    ln_b = np.ascontiguousarray(np.asarray(ln_b, dtype=np.float32))

    fold_ln = not np.any(ln_b)
    key = ("nc", fold_ln)
    if key not in _CACHE:
        _CACHE[key] = _build(fold_ln)
    nc = _CACHE[key]

    w_eff = w_qkv * ln_w[None, :] if fold_ln else w_qkv
    wT_qkv_h = np.ascontiguousarray(w_eff.T).astype(ml_dtypes.bfloat16)
    wT_out_h = np.ascontiguousarray(w_out.T).astype(ml_dtypes.bfloat16)
    in_maps = []
    for c in range(NCORES):
        xs = np.ascontiguousarray(
            x[c * SL:(c + 1) * SL].transpose(1, 0, 2).reshape(R, D))
        xsT = np.ascontiguousarray(xs.T)
        in_maps.append({
            "x_sh": xs, "xT_sh": xsT,
            "xb_sh": xsT.astype(ml_dtypes.bfloat16),
            "wT_qkv": wT_qkv_h, "wT_out": wT_out_h,
            "ln_w": ln_w, "ln_b": ln_b,
        })

    res = run_bass_kernel_spmd(nc, in_maps, list(range(NCORES)), trace=_trace,
                               tmpdir=_tmpdir)
    shards = [res.results[c]["out_sh"].reshape(B, SL, D).transpose(1, 0, 2)
              for c in range(NCORES)]
    out = np.concatenate(shards, axis=0)
    if _trace:
        _CACHE["last_result"] = res
    return out

